# revision 14
# baseline (speedup 1.0000x reference)
"""Causal shaped attention kernel for Trainium2 (8 NeuronCores).

y = beta * softmax(causal(q k^T / 8)) @ v + alpha * Id @ v - gamma * MC @ v
  with q,k = x @ w_attn.T split, v = x, Id = softmax(eye(T)), MC = causal row-mean.

Sharding: (batch, head-group) across 8 cores: core c -> b = c//2, heads
h0 = (c%2)*8 .. h0+8.  Each core computes y[b, :, h0*64 : h0*64+512].

Id@v and MC@v have closed forms (no T x T materialization):
  Id@v[i] = ((e-1) v[i] + colsum(v)) / (e+T-1)
  MC@v[i] = cumsum(v)[i] / (i+1)

On-device layout (per core):
  xT    [128, 8, 2048] bf16  x[b]^T by 128-wide c-chunks (PE-transposed)
  WTq   [128, 4, 8, 128] bf16  per head-pair p, c-chunk ci: [Wq_even^T | Wq_odd^T]
  WTk   same for k
  qkT   [128, 4, 2, 2048] bf16  pair p: partitions 0:64 even head, 64:128 odd;
                          [.., 0, :] = q^T, [.., 1, :] = k^T
  vload [128, 16, 512] f32   v rows by 128-tile (B2 colsum/cumsum operand)
  vones [128, 8, 16, 65] bf16  per head hh, j-tile J: [v | 1]  (AV lhsT)
  static [128, 16, 512] f32  k1*v + k2*colsum - gamma*cumsum/(i+1) addend

Attention per (pair p, i-strip g of 512): even/odd heads' S^T j-tiles go to the
two banks of one PSUM tile via concurrent K=64 row-group matmuls
(tile_position (0,0)/(64,0)).  Causal masking is an additive -2400*triu
matmul accumulated into PSUM before the scores (exp scale 1/8 -> -300 -> 0).
One exp (ACT) per (J, both heads) -> bf16 pt.  AV per head: lhsT=[v|1] bf16
gives y^T and rowsum in one accumulation chain; PE transpose back (bf16),
normalize + add static, DMA out.
"""

import sys

if "/opt/trn_rl_repo" not in sys.path:
    sys.path.insert(0, "/opt/trn_rl_repo")

import math

import numpy as np

import concourse.bass as bass
import concourse.mybir as mybir
import concourse.tile as tile
from concourse import bacc
from concourse.bass_utils import run_bass_kernel_spmd

F32 = mybir.dt.float32
F32R = mybir.dt.float32r
BF16 = mybir.dt.bfloat16
AF = mybir.ActivationFunctionType
OP = mybir.AluOpType

N_CORES = 8
B, T, C = 4, 2048, 1024
H, HD = 16, 64
NHC = 8          # heads per core
NT = T // 128    # 16 j/i tiles
NS = 4           # i-strips of 512
# consts: 128 tril + 16 negipg + k1 + k2 + kb + pad + 128 ident + 128 triuneg
CONSTS_W = 404

_NC_CACHE = {}


def r(ap):
    return ap.bitcast(F32R)


def emit(nc, tc, xb, wqk, consts, yout):
    ctx_pools = []

    def pool(name, **kw):
        p = tc.alloc_tile_pool(name=name, **kw)
        ctx_pools.append(p)
        return p

    cpool = pool("cpool", bufs=1)
    ps = pool("ps", bufs=2, space="PSUM")

    cons = cpool.tile([128, CONSTS_W], F32, name="cons")
    nc.sync.dma_start(out=cons[:], in_=consts[:])
    tril = cons[:, 0:128]
    ident = cons[:, 148:276]
    triuneg = cons[:, 276:404]
    negipg = cons[:, 128:144]      # [128, 16] : -gamma/(i+1)
    k1c = cons[:, 144:145]
    k2c = cons[:, 145:146]
    kbc = cons[:, 146:147]
    trilb = cpool.tile([128, 128], BF16, name="trilb")
    nc.vector.tensor_copy(out=trilb[:], in_=tril)
    ones_row = trilb[0:1, 0:128]   # tril row 0 == all ones (K=1 lhsT)
    ones_col = trilb[:, 127:128]   # tril col 127 == all ones [128, 1]
    identb = cpool.tile([128, 128], BF16, name="identb")
    nc.vector.tensor_copy(out=identb[:], in_=ident)
    triunegb = cpool.tile([128, 128], BF16, name="triunegb")
    nc.vector.tensor_copy(out=triunegb[:], in_=triuneg)

    qkp = pool("qkp", bufs=1)
    qkT = qkp.tile([128, 4, 2, 2048], BF16, name="qkT")

    # ---------------- phase A: transposes of W and x (fp32r, evac-cast bf16) --
    wtp = pool("wtp", bufs=1)
    WTq = wtp.tile([128, 4, 8, 128], BF16, name="WTq")
    WTk = wtp.tile([128, 4, 8, 128], BF16, name="WTk")
    xT = wtp.tile([128, 8, 2048], BF16, name="xT")

    ldp = pool("ldp", bufs=2)
    for p in range(4):
        for qk, WT in ((0, WTq), (1, WTk)):
            tw = ldp.tile([128, 1024], F32, name="tw", tag="tw")
            nc.sync.dma_start(out=tw[:], in_=wqk[qk * 512 + p * 128: qk * 512 + (p + 1) * 128, :])
            for cg in range(2):  # groups of 4 c-chunks
                pst = ps.tile([128, 512], F32, name="pst", tag="ps")
                for k in range(4):
                    ci = cg * 4 + k
                    nc.tensor.transpose(pst[:, k * 128:(k + 1) * 128],
                                        tw[:, ci * 128:(ci + 1) * 128], ident)
                nc.scalar.copy(out=WT[:, p, cg * 4:(cg + 1) * 4, :], in_=pst[:])
    for tt in range(NT):
        tx = ldp.tile([128, 1024], F32, name="tx", tag="tx")
        nc.sync.dma_start(out=tx[:], in_=xb[tt * 128:(tt + 1) * 128, :])
        for cg in range(2):
            pst = ps.tile([128, 512], F32, name="pstx", tag="ps")
            for k in range(4):
                ci = cg * 4 + k
                nc.tensor.transpose(pst[:, k * 128:(k + 1) * 128],
                                    tx[:, ci * 128:(ci + 1) * 128], ident)
            nc.vector.tensor_copy(out=xT[:, cg * 4:(cg + 1) * 4, tt * 128:(tt + 1) * 128],
                                  in_=pst[:].rearrange("p (a b) -> p a b", a=4))

    # ---------------- phase B: projections -> qkT (bf16 matmuls) ----------------
    for p in range(4):
        for qk, WT in ((0, WTq), (1, WTk)):
            for s in range(NS):
                pj = ps.tile([128, 512], F32, name="pj", tag="ps")
                for ci in range(8):
                    nc.tensor.matmul(pj[:], WT[:, p, ci, :],
                                     xT[:, ci, s * 512:(s + 1) * 512],
                                     start=(ci == 0), stop=(ci == 7))
                nc.scalar.copy(out=qkT[:, p, qk, s * 512:(s + 1) * 512], in_=pj[:])

    # ---------------- phase B2: vones, colsum/cumsum, static ----------------
    ldp.release()
    ctx_pools.remove(ldp)
    wtp.release()
    ctx_pools.remove(wtp)
    b2 = pool("b2", bufs=1)
    vload = b2.tile([128, NT, 512], F32, name="vload")
    nc.sync.dma_start(out=vload[:],
                      in_=xb[:, 0:512].rearrange("(J p) d -> p J d", p=128))
    vb16 = b2.tile([128, NT, 512], BF16, name="vb16")
    nc.vector.tensor_copy(out=vb16[:], in_=vload[:])
    vones = b2.tile([128, NHC, NT, 65], BF16, name="vones")
    nc.vector.memset(vones[:], 1.0)
    for hh in range(NHC):
        nc.vector.tensor_copy(out=vones[:, hh, :, 0:64],
                              in_=vb16[:, :, hh * 64:(hh + 1) * 64])

    colb = b2.tile([128, 512], F32, name="colb")
    run = b2.tile([1, 512], BF16, name="run")       # exclusive prefix of tile colsums
    runs = b2.tile([1, 512], F32, name="runs")      # k2-scaled total (staging)
    static = b2.tile([128, NT, 512], F32, name="static")

    # pass 1: total colsum -> colb
    cp1 = ps.tile([1, 512], F32, name="cp1", tag="cs", bufs=1)
    for I in range(NT):
        nc.tensor.matmul(cp1[0:1, :], ones_col, vb16[:, I, :],
                         start=(I == 0), stop=(I == NT - 1))
    nc.vector.tensor_scalar(out=runs[:], in0=cp1[0:1, :],
                            scalar1=cons[0:1, 145:146], scalar2=None, op0=OP.mult)
    nc.gpsimd.partition_broadcast(colb[:], runs[0:1, :])

    # pass 2: running exclusive prefix + cumsum + static
    nc.vector.memset(run[:], 0.0)
    for I in range(NT):
        cu = ps.tile([128, 512], F32, name="cu", tag="ps")
        nc.tensor.matmul(cu[:], ones_row, run[0:1, :], start=True, stop=False)
        nc.tensor.matmul(cu[:], trilb[:], vb16[:, I, :], start=False, stop=True)
        cp = ps.tile([1, 512], F32, name="cp2", tag="cs", bufs=1)
        nc.tensor.matmul(cp[0:1, :], ones_col, vb16[:, I, :],
                         start=True, stop=True)
        nc.vector.tensor_add(run[0:1, :], run[0:1, :], cp[0:1, :])
        nc.vector.scalar_tensor_tensor(
            out=static[:, I, :].rearrange("p (h d) -> p h d", h=NHC),
            in0=vload[:, I, :].rearrange("p (h d) -> p h d", h=NHC),
            scalar=k1c, in1=colb[:].rearrange("p (h d) -> p h d", h=NHC),
            op0=OP.mult, op1=OP.add)
        nc.vector.scalar_tensor_tensor(
            out=static[:, I, :], in0=cu[:], scalar=negipg[:, I:I + 1],
            in1=static[:, I, :], op0=OP.mult, op1=OP.add)

    # ---------------- phase C: attention, software-pipelined ----------------
    # Per strip s=(p,g): scores+exp emitted in groups of 2 j-tiles (one 4-bank
    # PSUM tile, exp over both heads); the PREVIOUS strip's AV matmuls and
    # y-post are interleaved between groups so the in-order PE has work while
    # ACT runs exp.  Steady state is ACT(exp)-bound with PE ~90% fed.
    ps.release()
    ctx_pools.remove(ps)
    cp3 = pool("cp3", bufs=1)
    # pt[buf][:, h, J, :] : exp(S^T) for head h (0=even,1=odd), j-tile J
    ptbuf = [cp3.tile([128, 2, NT, 512], BF16, name=f"pt{i}") for i in range(2)]
    ysp = pool("ysp", bufs=2)
    psC = pool("psC", bufs=1, space="PSUM")
    psY = pool("psY", bufs=1, space="PSUM")

    def mk_scores(p, g, pt):
        """Thunks, one per score-group of 2 consecutive j-tiles."""
        nj = 4 * g + 4
        qe = qkT[0:64, p, 0, :]
        qo = qkT[64:128, p, 0, :]
        ke = qkT[0:64, p, 1, :]
        ko = qkT[64:128, p, 1, :]
        gs = slice(g * 512, (g + 1) * 512)

        def grp(J0):
            st4 = psC.tile([128, 2, 2, 512], F32, name="st4", tag="st4", bufs=1)
            for jl, J in enumerate((J0, J0 + 1)):
                js = slice(J * 128, (J + 1) * 128)
                if J < 4 * g:
                    # full tile: concurrent K=64 row-group matmuls (even rows
                    # 0:64 of the PE array, odd rows 64:128)
                    nc.tensor.matmul(st4[:, 0, jl, :], ke[:, js], qe[:, gs],
                                     start=True, stop=True, tile_position=(0, 0),
                                     skip_group_check=True)
                    nc.tensor.matmul(st4[:, 1, jl, :], ko[:, js], qo[:, gs],
                                     start=True, stop=True, tile_position=(64, 0),
                                     skip_group_check=True)
                else:
                    # diagonal-region tile: -2400*triu written first (exp
                    # scale 1/8 -> -300 -> exp == 0), scores accumulate on top
                    i_off = 128 * J - 512 * g
                    blk = slice(i_off, i_off + 128)
                    gb = slice(g * 512 + i_off, g * 512 + i_off + 128)
                    for h, kk, qq, tpos in ((0, ke, qe, (0, 0)),
                                            (1, ko, qo, (64, 0))):
                        nc.tensor.matmul(st4[:, h, jl, blk], identb[:],
                                         triunegb[:], start=True, stop=False,
                                         skip_group_check=True)
                        nc.tensor.matmul(st4[:, h, jl, blk], kk[:, js], qq[:, gb],
                                         start=False, stop=True, tile_position=tpos,
                                         skip_group_check=True)
                        if i_off + 128 < 512:
                            rest = slice(i_off + 128, 512)
                            gr = slice(g * 512 + i_off + 128, (g + 1) * 512)
                            nc.tensor.matmul(st4[:, h, jl, rest], kk[:, js],
                                             qq[:, gr], start=True, stop=True,
                                             tile_position=tpos,
                                             skip_group_check=True)
            if J0 + 1 < 4 * g:
                nc.scalar.activation(out=pt[:, :, J0:J0 + 2, :], in_=st4[:],
                                     func=AF.Exp, scale=0.125)
            else:
                for jl, J in enumerate((J0, J0 + 1)):
                    io = max(0, 128 * J - 512 * g)
                    nc.scalar.activation(out=pt[:, :, J, io:512],
                                         in_=st4[:, :, jl, io:512],
                                         func=AF.Exp, scale=0.125)
        return [(lambda J0=J0: grp(J0)) for J0 in range(0, nj, 2)]

    def mk_avpost(p, g, pt):
        """Thunks: AV matmuls (J-major, halves interleaved) then y-post x2."""
        nj = 4 * g + 4
        state = {}

        def av(J, half):
            if half not in state:
                state[half] = psY.tile([128, 512], F32, name=f"yps{half}",
                                       tag=f"yps{half}", bufs=1)
            hh = 2 * p + half
            i_off = max(0, 128 * J - 512 * g)
            nc.tensor.matmul(state[half][0:65, i_off:512], vones[:, hh, J, :],
                             pt[:, half, J, i_off:512],
                             start=(J == 0), stop=(J == nj - 1),
                             skip_group_check=True)

        def ypost(half):
            yps = state[half]
            hh = 2 * p + half
            ysb = ysp.tile([65, 512], BF16, name="ysb", tag="ysb")
            nc.vector.tensor_copy(out=ysb[:], in_=yps[0:65, :])
            tp = psY.tile([128, 4, 66], BF16, name="tp", tag="tp", bufs=2)
            for k in range(4):
                nc.tensor.transpose(tp[:, k, 0:65],
                                    ysb[:, k * 128:(k + 1) * 128],
                                    identb[0:65, 0:65])
            rc4 = ysp.tile([128, 4], F32, name="rc4", tag="rc4")
            nc.vector.reciprocal(out=rc4[:], in_=tp[:, :, 64])
            nc.vector.tensor_scalar(out=rc4[:], in0=rc4[:], scalar1=kbc,
                                    scalar2=None, op0=OP.mult)
            yo = ysp.tile([128, 4, 64], F32, name="yo", tag="yo")
            for k in range(4):
                nc.vector.scalar_tensor_tensor(
                    out=yo[:, k, :], in0=tp[:, k, 0:64],
                    scalar=rc4[:, k:k + 1],
                    in1=static[:, 4 * g + k, hh * 64:(hh + 1) * 64],
                    op0=OP.mult, op1=OP.add)
            nc.sync.dma_start(
                out=yout[g * 512:(g + 1) * 512, hh * 64:(hh + 1) * 64]
                .rearrange("(k p) d -> p k d", p=128),
                in_=yo[:])

        thunks = [(lambda J=J, half=half: av(J, half))
                  for J in range(nj) for half in range(2)]
        thunks += [(lambda half=half: ypost(half)) for half in range(2)]
        return thunks

    strips = [(p, g) for p in range(4) for g in range(NS)]
    prev_work = []
    for si, (p, g) in enumerate(strips):
        pt = ptbuf[si % 2]
        sgroups = mk_scores(p, g, pt)
        m = len(sgroups)
        # distribute prev strip's AV/y-post work evenly between score groups
        w = len(prev_work)
        for k, sg in enumerate(sgroups):
            sg()
            lo, hi = (k * w) // m, ((k + 1) * w) // m
            for t in prev_work[lo:hi]:
                t()
        prev_work = mk_avpost(p, g, pt)
    for t in prev_work:
        t()

    for p in reversed(ctx_pools):
        p.release()


def build_nc():
    if "nc" in _NC_CACHE:
        return _NC_CACHE["nc"]
    nc = bacc.Bacc("TRN2", target_bir_lowering=False)
    xb = nc.declare_dram_parameter("xb", [T, C], F32, isOutput=False)
    wqk = nc.declare_dram_parameter("wqk", [C, C], F32, isOutput=False)
    consts = nc.declare_dram_parameter("consts", [128, CONSTS_W], F32, isOutput=False)
    yout = nc.declare_dram_parameter("yout", [T, 512], F32, isOutput=True)
    with tile.TileContext(nc) as tc:
        emit(nc, tc, xb, wqk, consts, yout)
    nc.compile()
    _NC_CACHE["nc"] = nc
    return nc


def make_consts(alpha, beta, gamma):
    D = math.e + T - 1
    k1 = alpha * (math.e - 1.0) / D
    k2 = alpha / D
    cons = np.zeros((128, CONSTS_W), dtype=np.float32)
    jj = np.arange(128)
    cons[:, 0:128] = (jj[:, None] <= jj[None, :]).astype(np.float32)  # tril mask
    for I in range(16):
        cons[:, 128 + I] = -gamma / (128.0 * I + jj + 1.0)
    cons[:, 144] = k1
    cons[:, 145] = k2
    cons[:, 146] = beta
    cons[:, 148:276] = np.eye(128, dtype=np.float32)
    # strict upper triangle (j > i): -2400 (exp scale 1/8 -> -300 -> exp = 0)
    cons[:, 276:404] = np.where(jj[:, None] > jj[None, :], -2400.0, 0.0)
    return cons


def kernel(x, w_attn, alpha, beta, gamma, _trace=False, _tmpdir=None):
    x = np.asarray(x, dtype=np.float32)
    w_attn = np.asarray(w_attn, dtype=np.float32)
    alpha = float(np.asarray(alpha))
    beta = float(np.asarray(beta))
    gamma = float(np.asarray(gamma))

    nc = build_nc()
    cons = make_consts(alpha, beta, gamma)
    in_maps = []
    for c in range(N_CORES):
        b, h0 = c // 2, (c % 2) * 8
        wqk = np.concatenate(
            [w_attn[h0 * 64: h0 * 64 + 512], w_attn[C + h0 * 64: C + h0 * 64 + 512]], axis=0)
        # rotate columns of x and w so this core's v-block sits at columns 0:512
        # (the projection q,k = x @ w.T is invariant to a consistent column roll)
        c0 = h0 * 64
        xb_r = np.roll(x[b], -c0, axis=1)
        wqk_r = np.roll(wqk, -c0, axis=1)
        in_maps.append({"xb": np.ascontiguousarray(xb_r),
                        "wqk": np.ascontiguousarray(wqk_r), "consts": cons})
    res = run_bass_kernel_spmd(nc, in_maps, list(range(N_CORES)), trace=_trace,
                               tmpdir=_tmpdir)
    y = np.empty((B, T, C), dtype=np.float32)
    for c in range(N_CORES):
        b, h0 = c // 2, (c % 2) * 8
        y[b, :, h0 * 64: h0 * 64 + 512] = res.results[c]["yout"]
    if _trace:
        kernel.last_exec_time_ns = res.exec_time_ns
    return y
